# revision 31
# baseline (speedup 1.0000x reference)
"""Trainium2 Bass kernel for nn_DiagonalRefine (8-core SPMD).

Math: the reference extracts the main diagonal of feat [2,256,512,512],
runs grouped-conv1d(k=3,g=8)+GELU, dense-conv1d(k=3)+GELU on it, embeds
the result back on the diagonal of a zero image, then depthwise 3x3-blurs
it. The blur of a diagonal-only image is zero outside 5 diagonals:
  out[i, i+d] for d in [-2..2], built from 9 per-channel blur weights and
  sig[i-1], sig[i], sig[i+1].

Sharding: rows are split 8 ways (64 rows/core). The host pre-gathers the
diagonal neighborhood (70 values per (b,c)); weights+diag arrive as bf16
(convs run as bf16 PE matmuls with both batches fused into one rhs),
blur coefficients/biases as f32. Exact GELU on ScalarE; band construction
on VectorE (combines) + ScalarE (pure scales, Copy-with-scale) into a
[h][d][b][i] tile that is the ONLY device output (655 KB/core): it holds
every nonzero of the result. Four band DMAs split by (h, writing engine)
so each trigger carries a single sync wait. The host unshard zero-fills
the full [2,256,512,512] tensor, places the 5 diagonals with strided
assignments, and applies an exact linear edge correction at the 2x2
corner blocks (the device runs unmasked; out-of-range conv taps at the
global edges are reproduced on the host from the same inputs and
subtracted — the band is linear in sig, so the fix is exact).

Wait-slot note: each instruction carries a single HW sync-wait slot, so
dummy observer ops watch each const DMA's semaphore on PE/ACT/DVE before
any real consumer needs it.
"""

import sys

for _p in ("/opt/trn_rl_repo",):
    if _p not in sys.path:
        sys.path.append(_p)

import ml_dtypes
import numpy as np
from scipy.special import erf

import concourse.bass as bass
import concourse.mybir as mybir
from concourse import tile
from concourse.bass_utils import run_bass_kernel_spmd
from bass_rust import add_dep_helper

# ---- problem geometry (hardcoded; see spec) --------------------------------
B = 2
C = 256
L = 512
NCORES = 8
RB = L // NCORES          # 64 rows per core
T = RB + 6                # 70 diag positions (halo 3 each side)
M = T - 2                 # 68 mid positions
S = M - 2                 # 66 sig positions
ND = 5                    # band diagonals per row: d-2..d+2
BRB = B * RB              # 128: elems per (h, d) band region
HB = ND * BRB             # 640: elems per h
BAND_ELEMS = 2 * HB * 128  # 163,840 elems (655 KB) per core
FP32 = mybir.dt.float32
BF16 = mybir.dt.bfloat16

# bf16 table per-partition layout (col offsets)
W1_OFF = 0                 # [6*128]  (k,h) -> [128ci_l, 128co_l] slab
W2_OFF = 6 * 128           # [12*128] (k,ci_h,h) -> [128, 128] slab
DG_OFF = W2_OFF + 12 * 128  # [2*2*T] diag [h][b][T]
WBH_OFF = DG_OFF + 4 * T   # [18] blur coeffs (h, ki*3+kj), bf16
CH_FREE = WBH_OFF + 18     # 2602

# f32 table per-partition layout
WB_OFF = 0                 # [18]  (h, ki*3+kj)
B1_OFF = 18                # [2]
B2_OFF = 20                # [2]
CT_FREE = 22

_cache = {}


def _build_nc(act=mybir.ActivationFunctionType.Gelu):
    nc = bass.Bass()
    wtabh = nc.declare_dram_parameter("wtabh", [128 * CH_FREE], BF16, isOutput=False)
    wtab = nc.declare_dram_parameter("wtab", [128 * CT_FREE], FP32, isOutput=False)
    outb = nc.declare_dram_parameter("out_band", [BAND_ELEMS], BF16, isOutput=True)

    mul = mybir.AluOpType.mult
    add = mybir.AluOpType.add

    with tile.TileContext(nc) as tc:
        with (
            tc.tile_pool(name="const", bufs=1) as cpool,
            tc.tile_pool(name="work", bufs=4) as wpool,
            tc.tile_pool(name="band", bufs=1) as bpool,
            tc.tile_pool(name="mpsum", bufs=2, space=bass.MemorySpace.PSUM) as mpool,
            tc.tile_pool(name="spsum", bufs=2, space=bass.MemorySpace.PSUM) as spool,
        ):
            # ---- const DMAs, split across both HWDGE rings so w1/diag land
            # first and conv1 starts while w2 streams in behind it.
            chtile = cpool.tile([128, CH_FREE], BF16, tag="chtile")
            ctile = cpool.tile([128, CT_FREE], FP32, tag="ctile")
            hdma1 = nc.scalar.dma_start(
                bass.AP(chtile.tensor, W1_OFF, [[CH_FREE, 128], [1, W2_OFF]]),
                bass.AP(wtabh, W1_OFF, [[CH_FREE, 128], [1, W2_OFF]]),
            )
            hdma2 = nc.scalar.dma_start(
                bass.AP(chtile.tensor, W2_OFF, [[CH_FREE, 128], [1, DG_OFF - W2_OFF]]),
                bass.AP(wtabh, W2_OFF, [[CH_FREE, 128], [1, DG_OFF - W2_OFF]]),
            )
            cdma = nc.sync.dma_start(
                ctile[:], bass.AP(wtab, 0, [[CT_FREE, 128], [1, CT_FREE]])
            )
            hdma3 = nc.sync.dma_start(
                bass.AP(chtile.tensor, DG_OFF, [[CH_FREE, 128], [1, CH_FREE - DG_OFF]]),
                bass.AP(wtabh, DG_OFF, [[CH_FREE, 128], [1, CH_FREE - DG_OFF]]),
            )

            # PSUM: conv1 out [128, 2M] per h, conv2 out [128, 2S] per h
            mps = [mpool.tile([128, 2 * M], FP32, tag="mps", name=f"mps{i}") for i in range(2)]
            sps = [spool.tile([128, 2 * S], FP32, tag="sps", name=f"sps{i}") for i in range(2)]

            # observer ops: each engine sees the const DMA semaphores before
            # any real consumer, keeping later ops at <=1 sync wait.
            scr = cpool.tile([1, 1], FP32, tag="scr")
            scr2 = cpool.tile([1, 1], FP32, tag="scr2")
            with tc.high_priority():
                nc.tensor.matmul(mps[0][0:2, 0:2], chtile[:, 0:2], chtile[:, 0:2],
                                 start=True, stop=True, skip_group_check=True)
                nc.scalar.copy(scr[:], ctile[0:1, 0:1])
                nc.vector.tensor_copy(scr2[:], ctile[0:1, 0:1])

            def w1slab(k, h):
                s = W1_OFF + (k * 2 + h) * 128
                return chtile[:, s:s + 128]

            def w2slab(k, ci_h, h):
                s = W2_OFF + ((k * 2 + ci_h) * 2 + h) * 128
                return chtile[:, s:s + 128]

            # band tile: [h][d][b][i]; every elementwise write is a
            # contiguous [128, 128] run.
            bandall = bpool.tile([128, 2 * HB], BF16, tag="bandall")

            # ---- conv1 (grouped, k=3), both batches fused in the rhs -----
            diag2 = [
                chtile[:, DG_OFF + h * B * T:DG_OFF + (h + 1) * B * T]
                .rearrange("p (b t) -> p b t", b=B)
                for h in range(2)
            ]
            hsb = []
            for h in range(2):
                mp = mps[h]
                for k in range(3):
                    nc.tensor.matmul(
                        mp[:], w1slab(k, h), diag2[h][:, :, k:k + M],
                        start=(k == 0), stop=(k == 2),
                        skip_group_check=(h == 0),
                    )
                hcur = wpool.tile([128, 2 * M], BF16, tag=f"h{h}", name=f"h{h}")
                nc.scalar.activation(
                    hcur[:], mp[:], act,
                    bias=ctile[:, B1_OFF + h:B1_OFF + h + 1],
                )
                hsb.append(hcur)

            # PE observer for the w2 DMA so conv2 matmuls keep <=1 sync wait
            nc.tensor.matmul(sps[0][0:2, 0:2], chtile[:, W2_OFF:W2_OFF + 2],
                             chtile[:, W2_OFF:W2_OFF + 2],
                             start=True, stop=True, skip_group_check=True)

            # ---- conv2 (dense, k=3): ci-half-major so the ci_h=0 taps run
            # while gelu1(h=1) is still producing the other half.
            sigs = []
            for h in range(2):
                sp = sps[h]
                # batch-minor interleaved psum layout: col = s*B + b, so the
                # band phase reads fully contiguous [128, 128] runs.
                sp_il = sp.rearrange("p (s b) -> p b s", b=B)
                for ci_h in range(2):
                    hs3 = hsb[ci_h].rearrange("p (b m) -> p b m", b=B)
                    for k in range(3):
                        last_mm = nc.tensor.matmul(
                            sp_il, w2slab(k, ci_h, h), hs3[:, :, k:k + S],
                            start=(ci_h == 0 and k == 0),
                            stop=(ci_h == 1 and k == 2),
                            skip_group_check=(ci_h == 0 and k == 0 and h == 0),
                        )
                sig = wpool.tile([128, 2 * S], BF16, tag=f"sig{h}", name=f"sig{h}")
                last_gelu = nc.scalar.activation(
                    sig[:], sp[:], act,
                    bias=ctile[:, B2_OFF + h:B2_OFF + h + 1],
                )
                sigs.append(sig)

            # ---- band construction + per-(h, engine) output DMAs ---------
            band_dmas = []
            act_bv4 = None
            for h in range(2):
                sig_h = sigs[h]

                def bv(d):
                    s = (h * ND + d) * BRB
                    return bandall[:, s:s + BRB]

                def sg(shift):
                    return sig_h[:, B * shift:B * shift + BRB]

                def wb(ki, kj):
                    s = WB_OFF + h * 9 + ki * 3 + kj
                    return ctile[:, s:s + 1]

                tmpA = bpool.tile([128, BRB], BF16, tag=f"tmpA{h}", name=f"tmpA{h}")
                tmpB = bpool.tile([128, BRB], BF16, tag=f"tmpB{h}", name=f"tmpB{h}")
                tmpC = bpool.tile([128, BRB], BF16, tag=f"tmpC{h}", name=f"tmpC{h}")
                tmpD = bpool.tile([128, BRB], BF16, tag=f"tmpD{h}", name=f"tmpD{h}")

                # ScalarE: pure-scale diagonals (Copy with per-partition scale)
                nc.scalar.mul(bv(0), sg(0), wb(0, 2))
                act_bv4 = nc.scalar.mul(bv(4), sg(2), wb(2, 0))
                # VectorE: products + combines
                nc.vector.tensor_scalar_mul(tmpA[:], sg(1), wb(1, 2))
                nc.vector.scalar_tensor_tensor(bv(1), sg(0), wb(0, 1), tmpA[:], mul, add)
                nc.vector.tensor_scalar_mul(tmpB[:], sg(2), wb(2, 1))
                dve_13 = nc.vector.scalar_tensor_tensor(bv(3), sg(1), wb(1, 0), tmpB[:], mul, add)
                nc.vector.tensor_scalar_mul(tmpC[:], sg(0), wb(0, 0))
                nc.vector.scalar_tensor_tensor(tmpD[:], sg(1), wb(1, 1), tmpC[:], mul, add)
                last_band = nc.vector.scalar_tensor_tensor(bv(2), sg(2), wb(2, 2), tmpD[:], mul, add)

                # VectorE band regions (d=1..3) -> one sync-ring DMA per h
                hb = h * HB
                band_dmas.append(nc.sync.dma_start(
                    bass.AP(outb, hb + BRB, [[2 * HB, 128], [1, 3 * BRB]]),
                    bass.AP(bandall.tensor, hb + BRB, [[2 * HB, 128], [1, 3 * BRB]]),
                ))

            # ScalarE band regions (d=0,4) per h on the scalar ring: same
            # engine as the writes, so the triggers need no sync wait.
            for h in range(2):
                hb = h * HB
                band_dmas.append(nc.scalar.dma_start(
                    bass.AP(outb, hb, [[2 * HB, 128], [4 * BRB, 2], [1, BRB]]),
                    bass.AP(bandall.tensor, hb, [[2 * HB, 128], [4 * BRB, 2], [1, BRB]]),
                ))

            # ---- tail nop ladders: bring each sequencer's observed clock
            # current one semaphore at a time so final drains need no
            # multi-waits.
            def ladder(eng, deps):
                for dinst in deps:
                    n = eng.nop()
                    add_dep_helper(n.ins, dinst.ins, reason="tail clock catch-up")
            alldeps = [hdma1, hdma2, hdma3, cdma, *band_dmas,
                       last_band, last_gelu, last_mm, act_bv4]
            for eng in (nc.sync, nc.scalar, nc.gpsimd, nc.vector, nc.tensor):
                ladder(eng, alldeps)
    return nc


def _prep_shared(w1, b1, w2, b2, w_blur):
    """Pack weights into the bf16 table [128, CH_FREE] and the f32 table
    [128, CT_FREE]; layouts along free dim documented at top of file."""
    chf = np.zeros((128, CH_FREE), np.float32)
    gc = C // 8
    for co in range(C):
        g = co // gc
        h, cil0 = divmod(g * gc, 128)
        co_l = co - h * 128
        for k in range(3):
            chf[cil0:cil0 + gc, W1_OFF + (k * 2 + h) * 128 + co_l] = w1[co, :, k]
    for k in range(3):
        for ci_h in range(2):
            for h in range(2):
                s = W2_OFF + ((k * 2 + ci_h) * 2 + h) * 128
                chf[:, s:s + 128] = w2[h * 128:(h + 1) * 128,
                                       ci_h * 128:(ci_h + 1) * 128, k].T
    chf[:, WBH_OFF:WBH_OFF + 18] = \
        w_blur.reshape(2, 128, 9).transpose(1, 0, 2).reshape(128, 18)
    ct = np.zeros((128, CT_FREE), np.float32)
    ct[:, WB_OFF:WB_OFF + 18] = \
        w_blur.reshape(2, 128, 9).transpose(1, 0, 2).reshape(128, 18)
    ct[:, B1_OFF:B1_OFF + 2] = b1.reshape(2, 128).T
    ct[:, B2_OFF:B2_OFF + 2] = b2.reshape(2, 128).T
    return chf, ct


def _gelu(x):
    return 0.5 * x * (1.0 + erf(x / np.sqrt(2.0)))


def _edge_fix(full, diag, w1, b1, w2, b2, w_blur):
    """The device computes unmasked: conv windows that extend past the
    global edges pick up GELU(bias)-style garbage instead of zero padding.
    Only sig at global positions {-1, 0, L-1, L} are affected, and the band
    is linear in sig, so replaying the device's edge math on the host gives
    an exact correction confined to the 2x2 corner blocks."""
    grp = np.arange(C) // (C // 8)          # group of each channel
    gbase = grp * (C // 8)
    cols = gbase[:, None] + np.arange(C // 8)[None, :]   # [C, 32]

    def hs_at(dwin):
        # dwin: list of 3 arrays [B, C] (or None = zero padding)
        pre = np.broadcast_to(b1, (B, C)).copy()
        for k, v in enumerate(dwin):
            if v is not None:
                pre = pre + (w1[None, :, :, k] * v[:, cols]).sum(2)
        return _gelu(pre)

    def sig_at(hwin):
        # hwin: list of 3 arrays [B, C] (or None)
        pre = np.broadcast_to(b2, (B, C)).copy()
        for k, v in enumerate(hwin):
            if v is not None:
                pre = pre + np.einsum('oc,bc->bo', w2[:, :, k], v)
        return _gelu(pre)

    d0, d1v = diag[:, :, 0], diag[:, :, 1]
    dLm1, dLm2, dLm3 = diag[:, :, L - 1], diag[:, :, L - 2], diag[:, :, L - 3]
    zero = np.zeros((B, C), np.float32)

    hsE0 = hs_at([None, None, None])          # gm = -2 and gm = L+1
    hsE1 = hs_at([None, None, d0])            # gm = -1
    hsEL = hs_at([dLm1, None, None])          # gm = L
    hsT0 = hs_at([None, d0, d1v])             # gm = 0 (true)
    hsT1 = hs_at([d0, d1v, diag[:, :, 2]])    # gm = 1 (true)
    hsTLm2 = hs_at([dLm3, dLm2, dLm1])        # gm = L-2 (true)
    hsTLm1 = hs_at([dLm2, dLm1, None])        # gm = L-1 (true)

    sig_dev0 = sig_at([hsE0, hsE1, hsT0])     # gs = -1 (device garbage)
    sig_dev1 = sig_at([hsE1, hsT0, hsT1])     # gs = 0 (device)
    sig_tru1 = sig_at([None, hsT0, hsT1])     # gs = 0 (true)
    sig_devR = sig_at([hsTLm2, hsTLm1, hsEL])  # gs = L-1 (device)
    sig_truR = sig_at([hsTLm2, hsTLm1, None])  # gs = L-1 (true)
    sig_devL = sig_at([hsTLm1, hsEL, hsE0])   # gs = L (device garbage)

    dB = -sig_dev0
    dA = sig_tru1 - sig_dev1
    dC = sig_truR - sig_devR
    dD = -sig_devL
    w = w_blur[:, 0]                          # [C, 3, 3]
    full[:, :, 0, 0] += w[None, :, 0, 0] * dB + w[None, :, 1, 1] * dA
    full[:, :, 1, 1] += w[None, :, 0, 0] * dA
    full[:, :, 1, 0] += w[None, :, 0, 1] * dA
    full[:, :, 0, 1] += w[None, :, 1, 0] * dA
    full[:, :, L - 1, L - 1] += w[None, :, 1, 1] * dC + w[None, :, 2, 2] * dD
    full[:, :, L - 1, L - 2] += w[None, :, 1, 2] * dC
    full[:, :, L - 2, L - 1] += w[None, :, 2, 1] * dC
    full[:, :, L - 2, L - 2] += w[None, :, 2, 2] * dC


def _run(inputs, trace=False, **kw):
    feat = np.asarray(inputs["feat"], np.float32)
    w1 = np.asarray(inputs["w1"], np.float32)
    b1 = np.asarray(inputs["b1"], np.float32)
    w2 = np.asarray(inputs["w2"], np.float32)
    b2 = np.asarray(inputs["b2"], np.float32)
    w_blur = np.asarray(inputs["w_blur"], np.float32)
    chf, ct = _prep_shared(w1, b1, w2, b2, w_blur)
    # host-side diagonal gather (tiny: [B,C,L] = 1 MiB), zero-padded halo
    diag = np.ascontiguousarray(np.diagonal(feat, axis1=2, axis2=3))  # [B,C,L]
    diagp = np.zeros((B, C, L + 6), np.float32)
    diagp[:, :, 3:L + 3] = diag
    ctr = ct.ravel()
    in_maps = []
    for g in range(NCORES):
        base = g * RB
        chg = chf.copy()
        for h in range(2):
            for b in range(B):
                o = DG_OFF + (h * B + b) * T
                chg[:, o:o + T] = diagp[b, h * 128:(h + 1) * 128, base:base + T]
        in_maps.append({"wtabh": chg.astype(ml_dtypes.bfloat16).ravel(),
                        "wtab": ctr})
    if "nc" not in _cache:
        _cache["nc"] = _build_nc()
    res = run_bass_kernel_spmd(
        _cache["nc"], in_maps, core_ids=list(range(NCORES)), trace=trace, **kw
    )
    _cache["last_result"] = res

    # unshard: zero-fill, then place the 5 diagonals with strided writes.
    # gband[b, c, d, i] = out[b, c, i, i+d-2]
    gband = np.empty((B, C, ND, L), np.float32)
    for g in range(NCORES):
        arr = np.asarray(res.results[g]["out_band"]).astype(np.float32)
        arr = arr.reshape(128, 2, ND, RB, B)
        gband[:, :, :, g * RB:(g + 1) * RB] = \
            arr.transpose(4, 1, 0, 2, 3).reshape(B, C, ND, RB)
    full = np.zeros((B, C, L, L), np.float32)
    flat = full.reshape(B, C, L * L)
    for dd in range(ND):
        d = dd - 2
        i0 = max(0, -d)
        cnt = L - abs(d)
        # row i, col i+d -> flat i*(L+1) + d
        flat[:, :, i0 * (L + 1) + d::L + 1][:, :, :cnt] = \
            gband[:, :, dd, i0:i0 + cnt]
    _edge_fix(full, diag, w1, b1, w2, b2, w_blur)
    return full


def kernel(**inputs):
    return _run(inputs, trace=False)


# revision 32
# speedup vs baseline: 1.0234x; 1.0234x over previous
"""Trainium2 Bass kernel for nn_DiagonalRefine (8-core SPMD).

Math: the reference extracts the main diagonal of feat [2,256,512,512],
runs grouped-conv1d(k=3,g=8)+GELU, dense-conv1d(k=3)+GELU on it, embeds
the result back on the diagonal of a zero image, then depthwise 3x3-blurs
it. The blur of a diagonal-only image is zero outside 5 diagonals:
  out[i, i+d] for d in [-2..2], built from 9 per-channel blur weights and
  sig[i-1], sig[i], sig[i+1].

Sharding: rows are split 8 ways (64 rows/core). The host pre-gathers the
diagonal neighborhood (70 values per (b,c)); weights+diag arrive as bf16
(convs run as bf16 PE matmuls with both batches fused into one rhs),
blur coefficients/biases as f32. Exact GELU on ScalarE; band construction
on VectorE (combines) + ScalarE (pure scales, Copy-with-scale) into a
[h][d][b][i] tile that is the ONLY device output (655 KB/core): it holds
every nonzero of the result. Four band DMAs split by (h, writing engine)
so each trigger carries a single sync wait. The host unshard zero-fills
the full [2,256,512,512] tensor, places the 5 diagonals with strided
assignments, and applies an exact linear edge correction at the 2x2
corner blocks (the device runs unmasked; out-of-range conv taps at the
global edges are reproduced on the host from the same inputs and
subtracted — the band is linear in sig, so the fix is exact).

Wait-slot note: each instruction carries a single HW sync-wait slot, so
dummy observer ops watch each const DMA's semaphore on PE/ACT/DVE before
any real consumer needs it.
"""

import sys

for _p in ("/opt/trn_rl_repo",):
    if _p not in sys.path:
        sys.path.append(_p)

import ml_dtypes
import numpy as np
from scipy.special import erf

import concourse.bass as bass
import concourse.mybir as mybir
from concourse import tile
from concourse.bass_utils import run_bass_kernel_spmd
from bass_rust import add_dep_helper

# ---- problem geometry (hardcoded; see spec) --------------------------------
B = 2
C = 256
L = 512
NCORES = 8
RB = L // NCORES          # 64 rows per core
T = RB + 6                # 70 diag positions (halo 3 each side)
M = T - 2                 # 68 mid positions
S = M - 2                 # 66 sig positions
ND = 5                    # band diagonals per row: d-2..d+2
BRB = B * RB              # 128: elems per (h, d) band region
HB = ND * BRB             # 640: elems per h
BAND_ELEMS = 2 * HB * 128  # 163,840 elems (655 KB) per core
FP32 = mybir.dt.float32
BF16 = mybir.dt.bfloat16

# bf16 table per-partition layout (col offsets)
W1_OFF = 0                 # [6*128]  (k,h) -> [128ci_l, 128co_l] slab
W2_OFF = 6 * 128           # [12*128] (k,ci_h,h) -> [128, 128] slab
DG_OFF = W2_OFF + 12 * 128  # [2*2*T] diag [h][b][T]
WBH_OFF = DG_OFF + 4 * T   # [18] blur coeffs (h, ki*3+kj), bf16
CH_FREE = WBH_OFF + 18     # 2602

# f32 table per-partition layout
WB_OFF = 0                 # [18]  (h, ki*3+kj)
B1_OFF = 18                # [2]
B2_OFF = 20                # [2]
CT_FREE = 22

_cache = {}


def _build_nc(act=mybir.ActivationFunctionType.Gelu):
    nc = bass.Bass()
    wtabh = nc.declare_dram_parameter("wtabh", [128 * CH_FREE], BF16, isOutput=False)
    wtab = nc.declare_dram_parameter("wtab", [128 * CT_FREE], FP32, isOutput=False)
    outb = nc.declare_dram_parameter("out_band", [BAND_ELEMS], BF16, isOutput=True)

    mul = mybir.AluOpType.mult
    add = mybir.AluOpType.add

    with tile.TileContext(nc) as tc:
        with (
            tc.tile_pool(name="const", bufs=1) as cpool,
            tc.tile_pool(name="work", bufs=4) as wpool,
            tc.tile_pool(name="band", bufs=1) as bpool,
            tc.tile_pool(name="mpsum", bufs=2, space=bass.MemorySpace.PSUM) as mpool,
            tc.tile_pool(name="spsum", bufs=2, space=bass.MemorySpace.PSUM) as spool,
        ):
            # ---- const DMAs, split across both HWDGE rings so w1/diag land
            # first and conv1 starts while w2 streams in behind it.
            chtile = cpool.tile([128, CH_FREE], BF16, tag="chtile")
            ctile = cpool.tile([128, CT_FREE], FP32, tag="ctile")
            hdma1 = nc.scalar.dma_start(
                bass.AP(chtile.tensor, W1_OFF, [[CH_FREE, 128], [1, W2_OFF]]),
                bass.AP(wtabh, W1_OFF, [[CH_FREE, 128], [1, W2_OFF]]),
            )
            hdma2 = nc.scalar.dma_start(
                bass.AP(chtile.tensor, W2_OFF, [[CH_FREE, 128], [1, DG_OFF - W2_OFF]]),
                bass.AP(wtabh, W2_OFF, [[CH_FREE, 128], [1, DG_OFF - W2_OFF]]),
            )
            cdma = nc.sync.dma_start(
                ctile[:], bass.AP(wtab, 0, [[CT_FREE, 128], [1, CT_FREE]])
            )
            hdma3 = nc.sync.dma_start(
                bass.AP(chtile.tensor, DG_OFF, [[CH_FREE, 128], [1, CH_FREE - DG_OFF]]),
                bass.AP(wtabh, DG_OFF, [[CH_FREE, 128], [1, CH_FREE - DG_OFF]]),
            )

            # PSUM: conv1 out [128, 2M] per h, conv2 out [128, 2S] per h
            mps = [mpool.tile([128, 2 * M], FP32, tag="mps", name=f"mps{i}") for i in range(2)]
            sps = [spool.tile([128, 2 * S], FP32, tag="sps", name=f"sps{i}") for i in range(2)]

            # observer ops: each engine sees the const DMA semaphores before
            # any real consumer, keeping later ops at <=1 sync wait.
            scr = cpool.tile([1, 1], FP32, tag="scr")
            scr2 = cpool.tile([1, 1], FP32, tag="scr2")
            with tc.high_priority():
                nc.tensor.matmul(mps[0][0:2, 0:2], chtile[:, 0:2], chtile[:, 0:2],
                                 start=True, stop=True, skip_group_check=True)
                nc.scalar.copy(scr[:], ctile[0:1, 0:1])
                nc.vector.tensor_copy(scr2[:], ctile[0:1, 0:1])

            def w1slab(k, h):
                s = W1_OFF + (k * 2 + h) * 128
                return chtile[:, s:s + 128]

            def w2slab(k, ci_h, h):
                s = W2_OFF + ((k * 2 + ci_h) * 2 + h) * 128
                return chtile[:, s:s + 128]

            # band tile: [h][d][b][i]; every elementwise write is a
            # contiguous [128, 128] run.
            bandall = bpool.tile([128, 2 * HB], BF16, tag="bandall")

            # ---- conv1 (grouped, k=3), both batches fused in the rhs -----
            diag2 = [
                chtile[:, DG_OFF + h * B * T:DG_OFF + (h + 1) * B * T]
                .rearrange("p (b t) -> p b t", b=B)
                for h in range(2)
            ]
            hsb = []
            for h in range(2):
                mp = mps[h]
                for k in range(3):
                    nc.tensor.matmul(
                        mp[:], w1slab(k, h), diag2[h][:, :, k:k + M],
                        start=(k == 0), stop=(k == 2),
                        skip_group_check=(h == 0),
                    )
                hcur = wpool.tile([128, 2 * M], BF16, tag=f"h{h}", name=f"h{h}")
                nc.scalar.activation(
                    hcur[:], mp[:], act,
                    bias=ctile[:, B1_OFF + h:B1_OFF + h + 1],
                )
                hsb.append(hcur)

            # PE observer for the w2 DMA so conv2 matmuls keep <=1 sync wait
            nc.tensor.matmul(sps[0][0:2, 0:2], chtile[:, W2_OFF:W2_OFF + 2],
                             chtile[:, W2_OFF:W2_OFF + 2],
                             start=True, stop=True, skip_group_check=True)

            # ---- conv2 (dense, k=3): ci-half-major so the ci_h=0 taps run
            # while gelu1(h=1) is still producing the other half.
            sigs = []
            for h in range(2):
                sp = sps[h]
                for ci_h in range(2):
                    hs3 = hsb[ci_h].rearrange("p (b m) -> p b m", b=B)
                    for k in range(3):
                        last_mm = nc.tensor.matmul(
                            sp[:], w2slab(k, ci_h, h), hs3[:, :, k:k + S],
                            start=(ci_h == 0 and k == 0),
                            stop=(ci_h == 1 and k == 2),
                            skip_group_check=(ci_h == 0 and k == 0 and h == 0),
                        )
                sig = wpool.tile([128, 2 * S], BF16, tag=f"sig{h}", name=f"sig{h}")
                last_gelu = nc.scalar.activation(
                    sig[:], sp[:], act,
                    bias=ctile[:, B2_OFF + h:B2_OFF + h + 1],
                )
                sigs.append(sig)

            # ---- band construction + per-(h, engine) output DMAs ---------
            band_dmas = []
            act_bv4 = None
            for h in range(2):
                sig3 = sigs[h].rearrange("p (b s) -> p b s", b=B)

                def bv(d):
                    s = (h * ND + d) * BRB
                    return bandall[:, s:s + BRB]

                def sg(shift):
                    return sig3[:, :, shift:shift + RB]

                def wb(ki, kj):
                    s = WB_OFF + h * 9 + ki * 3 + kj
                    return ctile[:, s:s + 1]

                tmpA = bpool.tile([128, BRB], BF16, tag=f"tmpA{h}", name=f"tmpA{h}")
                tmpB = bpool.tile([128, BRB], BF16, tag=f"tmpB{h}", name=f"tmpB{h}")
                tmpC = bpool.tile([128, BRB], BF16, tag=f"tmpC{h}", name=f"tmpC{h}")
                tmpD = bpool.tile([128, BRB], BF16, tag=f"tmpD{h}", name=f"tmpD{h}")

                # ScalarE: pure-scale diagonals (Copy with per-partition scale)
                nc.scalar.mul(bv(0), sg(0), wb(0, 2))
                act_bv4 = nc.scalar.mul(bv(4), sg(2), wb(2, 0))
                # VectorE: products + combines
                nc.vector.tensor_scalar_mul(tmpA[:], sg(1), wb(1, 2))
                nc.vector.scalar_tensor_tensor(bv(1), sg(0), wb(0, 1), tmpA[:], mul, add)
                nc.vector.tensor_scalar_mul(tmpB[:], sg(2), wb(2, 1))
                dve_13 = nc.vector.scalar_tensor_tensor(bv(3), sg(1), wb(1, 0), tmpB[:], mul, add)
                nc.vector.tensor_scalar_mul(tmpC[:], sg(0), wb(0, 0))
                nc.vector.scalar_tensor_tensor(tmpD[:], sg(1), wb(1, 1), tmpC[:], mul, add)
                last_band = nc.vector.scalar_tensor_tensor(bv(2), sg(2), wb(2, 2), tmpD[:], mul, add)

                # VectorE band regions (d=1..3) -> one sync-ring DMA per h
                hb = h * HB
                band_dmas.append(nc.sync.dma_start(
                    bass.AP(outb, hb + BRB, [[2 * HB, 128], [1, 3 * BRB]]),
                    bass.AP(bandall.tensor, hb + BRB, [[2 * HB, 128], [1, 3 * BRB]]),
                ))

            # ScalarE band regions (d=0,4) per h on the scalar ring: same
            # engine as the writes, so the triggers need no sync wait.
            for h in range(2):
                hb = h * HB
                band_dmas.append(nc.scalar.dma_start(
                    bass.AP(outb, hb, [[2 * HB, 128], [4 * BRB, 2], [1, BRB]]),
                    bass.AP(bandall.tensor, hb, [[2 * HB, 128], [4 * BRB, 2], [1, BRB]]),
                ))

            # ---- tail nop ladders: bring each sequencer's observed clock
            # current one semaphore at a time so final drains need no
            # multi-waits.
            def ladder(eng, deps):
                for dinst in deps:
                    n = eng.nop()
                    add_dep_helper(n.ins, dinst.ins, reason="tail clock catch-up")
            alldeps = [hdma1, hdma2, hdma3, cdma, *band_dmas,
                       last_band, last_gelu, last_mm, act_bv4]
            for eng in (nc.sync, nc.scalar, nc.gpsimd, nc.vector, nc.tensor):
                ladder(eng, alldeps)
    return nc


def _prep_shared(w1, b1, w2, b2, w_blur):
    """Pack weights into the bf16 table [128, CH_FREE] and the f32 table
    [128, CT_FREE]; layouts along free dim documented at top of file."""
    chf = np.zeros((128, CH_FREE), np.float32)
    gc = C // 8
    for co in range(C):
        g = co // gc
        h, cil0 = divmod(g * gc, 128)
        co_l = co - h * 128
        for k in range(3):
            chf[cil0:cil0 + gc, W1_OFF + (k * 2 + h) * 128 + co_l] = w1[co, :, k]
    for k in range(3):
        for ci_h in range(2):
            for h in range(2):
                s = W2_OFF + ((k * 2 + ci_h) * 2 + h) * 128
                chf[:, s:s + 128] = w2[h * 128:(h + 1) * 128,
                                       ci_h * 128:(ci_h + 1) * 128, k].T
    chf[:, WBH_OFF:WBH_OFF + 18] = \
        w_blur.reshape(2, 128, 9).transpose(1, 0, 2).reshape(128, 18)
    ct = np.zeros((128, CT_FREE), np.float32)
    ct[:, WB_OFF:WB_OFF + 18] = \
        w_blur.reshape(2, 128, 9).transpose(1, 0, 2).reshape(128, 18)
    ct[:, B1_OFF:B1_OFF + 2] = b1.reshape(2, 128).T
    ct[:, B2_OFF:B2_OFF + 2] = b2.reshape(2, 128).T
    return chf, ct


def _gelu(x):
    return 0.5 * x * (1.0 + erf(x / np.sqrt(2.0)))


def _edge_fix(full, diag, w1, b1, w2, b2, w_blur):
    """The device computes unmasked: conv windows that extend past the
    global edges pick up GELU(bias)-style garbage instead of zero padding.
    Only sig at global positions {-1, 0, L-1, L} are affected, and the band
    is linear in sig, so replaying the device's edge math on the host gives
    an exact correction confined to the 2x2 corner blocks."""
    grp = np.arange(C) // (C // 8)          # group of each channel
    gbase = grp * (C // 8)
    cols = gbase[:, None] + np.arange(C // 8)[None, :]   # [C, 32]

    def hs_at(dwin):
        # dwin: list of 3 arrays [B, C] (or None = zero padding)
        pre = np.broadcast_to(b1, (B, C)).copy()
        for k, v in enumerate(dwin):
            if v is not None:
                pre = pre + (w1[None, :, :, k] * v[:, cols]).sum(2)
        return _gelu(pre)

    def sig_at(hwin):
        # hwin: list of 3 arrays [B, C] (or None)
        pre = np.broadcast_to(b2, (B, C)).copy()
        for k, v in enumerate(hwin):
            if v is not None:
                pre = pre + np.einsum('oc,bc->bo', w2[:, :, k], v)
        return _gelu(pre)

    d0, d1v = diag[:, :, 0], diag[:, :, 1]
    dLm1, dLm2, dLm3 = diag[:, :, L - 1], diag[:, :, L - 2], diag[:, :, L - 3]
    zero = np.zeros((B, C), np.float32)

    hsE0 = hs_at([None, None, None])          # gm = -2 and gm = L+1
    hsE1 = hs_at([None, None, d0])            # gm = -1
    hsEL = hs_at([dLm1, None, None])          # gm = L
    hsT0 = hs_at([None, d0, d1v])             # gm = 0 (true)
    hsT1 = hs_at([d0, d1v, diag[:, :, 2]])    # gm = 1 (true)
    hsTLm2 = hs_at([dLm3, dLm2, dLm1])        # gm = L-2 (true)
    hsTLm1 = hs_at([dLm2, dLm1, None])        # gm = L-1 (true)

    sig_dev0 = sig_at([hsE0, hsE1, hsT0])     # gs = -1 (device garbage)
    sig_dev1 = sig_at([hsE1, hsT0, hsT1])     # gs = 0 (device)
    sig_tru1 = sig_at([None, hsT0, hsT1])     # gs = 0 (true)
    sig_devR = sig_at([hsTLm2, hsTLm1, hsEL])  # gs = L-1 (device)
    sig_truR = sig_at([hsTLm2, hsTLm1, None])  # gs = L-1 (true)
    sig_devL = sig_at([hsTLm1, hsEL, hsE0])   # gs = L (device garbage)

    dB = -sig_dev0
    dA = sig_tru1 - sig_dev1
    dC = sig_truR - sig_devR
    dD = -sig_devL
    w = w_blur[:, 0]                          # [C, 3, 3]
    full[:, :, 0, 0] += w[None, :, 0, 0] * dB + w[None, :, 1, 1] * dA
    full[:, :, 1, 1] += w[None, :, 0, 0] * dA
    full[:, :, 1, 0] += w[None, :, 0, 1] * dA
    full[:, :, 0, 1] += w[None, :, 1, 0] * dA
    full[:, :, L - 1, L - 1] += w[None, :, 1, 1] * dC + w[None, :, 2, 2] * dD
    full[:, :, L - 1, L - 2] += w[None, :, 1, 2] * dC
    full[:, :, L - 2, L - 1] += w[None, :, 2, 1] * dC
    full[:, :, L - 2, L - 2] += w[None, :, 2, 2] * dC


def _run(inputs, trace=False, **kw):
    feat = np.asarray(inputs["feat"], np.float32)
    w1 = np.asarray(inputs["w1"], np.float32)
    b1 = np.asarray(inputs["b1"], np.float32)
    w2 = np.asarray(inputs["w2"], np.float32)
    b2 = np.asarray(inputs["b2"], np.float32)
    w_blur = np.asarray(inputs["w_blur"], np.float32)
    chf, ct = _prep_shared(w1, b1, w2, b2, w_blur)
    # host-side diagonal gather (tiny: [B,C,L] = 1 MiB), zero-padded halo
    diag = np.ascontiguousarray(np.diagonal(feat, axis1=2, axis2=3))  # [B,C,L]
    diagp = np.zeros((B, C, L + 6), np.float32)
    diagp[:, :, 3:L + 3] = diag
    ctr = ct.ravel()
    in_maps = []
    for g in range(NCORES):
        base = g * RB
        chg = chf.copy()
        for h in range(2):
            for b in range(B):
                o = DG_OFF + (h * B + b) * T
                chg[:, o:o + T] = diagp[b, h * 128:(h + 1) * 128, base:base + T]
        in_maps.append({"wtabh": chg.astype(ml_dtypes.bfloat16).ravel(),
                        "wtab": ctr})
    if "nc" not in _cache:
        _cache["nc"] = _build_nc()
    res = run_bass_kernel_spmd(
        _cache["nc"], in_maps, core_ids=list(range(NCORES)), trace=trace, **kw
    )
    _cache["last_result"] = res

    # unshard: zero-fill, then place the 5 diagonals with strided writes.
    # gband[b, c, d, i] = out[b, c, i, i+d-2]
    gband = np.empty((B, C, ND, L), np.float32)
    for g in range(NCORES):
        arr = np.asarray(res.results[g]["out_band"]).astype(np.float32)
        arr = arr.reshape(128, 2, ND, B, RB)
        gband[:, :, :, g * RB:(g + 1) * RB] = \
            arr.transpose(3, 1, 0, 2, 4).reshape(B, C, ND, RB)
    full = np.zeros((B, C, L, L), np.float32)
    flat = full.reshape(B, C, L * L)
    for dd in range(ND):
        d = dd - 2
        i0 = max(0, -d)
        cnt = L - abs(d)
        # row i, col i+d -> flat i*(L+1) + d
        flat[:, :, i0 * (L + 1) + d::L + 1][:, :, :cnt] = \
            gband[:, :, dd, i0:i0 + cnt]
    _edge_fix(full, diag, w1, b1, w2, b2, w_blur)
    return full


def kernel(**inputs):
    return _run(inputs, trace=False)
